# revision 1
# baseline (speedup 1.0000x reference)
"""Trainium2 Bass kernel for nn_Discriminator (AdderNet CNN, 5 layers).

Per core (batch-sharded 256/8=32):
  adder2d(x,W) = -sum_d |p_d - w_d| = -S1 + SW + 2*M2
      S1 = sum_d p_d   (PE matmul, block-ones lhsT = -1.0, shared by all co)
      SW = sum_d w_d   (host constant, folded into ACT copy bias)
      M2 = sum_d min(p_d - w_d, 0)
           (DVE tensor_scalar (subtract,min), d on partitions, per-partition
            weight scalar; reduced over d by PE matmul with sliding one-hot
            lhsT = +2.0 into the psum row of the output channel)
  Training-mode BN: per-channel sum/sumsq via ACT accum_out, folded across
  psum rows by one-hot matmul, AllReduce [C,2] across 8 cores, scale/bias on
  device (sqrt + Newton + reciprocal), applied fused with LeakyReLU (Prelu).
  Layer-5 ends with Sigmoid. Patches for L2-L5 are strided ACT copies from
  zero-padded activation buffers; L1 patches (Ci=1) are im2col'd on host.
"""
import numpy as np
import ml_dtypes

NCORES = 8
NPC = 32
EPS = 1e-5
SLOPE = 0.2
BF = ml_dtypes.bfloat16

_cache = {}


def _install_bir_fix():
    """walrus workaround: ISA allows 1 sync-wait per instruction (2 for
    EventSemaphore); hoist excess waits onto injected EventSemaphores."""
    import orjson
    import concourse.bass_utils as bu
    import concourse.bass2jax as b2j

    if getattr(bu.compile_bir_kernel, "_waitfix", False):
        return

    def _fix(bir_json):
        bir = orjson.loads(bir_json)
        mods = bir.get("modules") or [bir]
        n = 0
        changed = False
        for mod in mods:
            for fn in mod.get("functions", []):
                for blk in fn.get("blocks", []):
                    out = []
                    for ins in blk.get("instructions", []):
                        cap = 2 if ins.get("opcode") == "EventSemaphore" else 1
                        waits = ins.get("sync_info", {}).get("on_wait", [])
                        if len(waits) > cap:
                            changed = True
                            for w in waits[:-cap]:
                                n += 1
                                out.append({
                                    "engine": ins["engine"], "ins": [], "outs": [],
                                    "name": f"I-waitfix-{n}",
                                    "opcode": "EventSemaphore",
                                    "sync_info": {"on_update": [], "on_wait": [w]},
                                    **({"debug": ins["debug"]} if "debug" in ins else {}),
                                })
                            ins["sync_info"]["on_wait"] = waits[-cap:]
                        out.append(ins)
                    blk["instructions"] = out
        return orjson.dumps(bir) if changed else bir_json

    orig = bu.compile_bir_kernel

    def wrapped(bir_json, tmpdir, neff_name="file.neff"):
        return orig(_fix(bir_json), tmpdir, neff_name)

    wrapped._waitfix = True
    bu.compile_bir_kernel = wrapped
    b2j.compile_bir_kernel = wrapped

    hook_orig = b2j.neuronx_cc_hook

    def hook_logged(*a, **k):
        try:
            return hook_orig(*a, **k)
        except BaseException:
            import traceback
            with open("/tmp/hook_err.log", "a") as f:
                f.write("=== neuronx_cc_hook failed ===\n")
                traceback.print_exc(file=f)
            raise

    b2j.neuronx_cc_hook = hook_logged


# layer geometry; d-order (kh, kw, ci); positions q = (n*Ho + ho)*Wo + wo
LCFG = {
    2: dict(Ci=16, Co=32, K=4, Ho=32, dparts=[128, 128], rowmod=32,
            npg=4, n_chunks=16, ck_n=2, rawW=8192, nst=16),
    3: dict(Ci=32, Co=64, K=3, Ho=16, dparts=[128, 128, 32], rowmod=64,
            npg=8, n_chunks=4, ck_n=8, rawW=4096, nst=8),
    4: dict(Ci=64, Co=128, K=4, Ho=8, dparts=[128] * 8, rowmod=128,
            npg=16, n_chunks=1, ck_n=32, rawW=2048, nst=4),
    5: dict(Ci=128, Co=1, K=4, Ho=4, dparts=[128] * 16, rowmod=128,
            npg=32, n_chunks=1, ck_n=32, rawW=512, nst=1),
}
CNT = {1: 256 * 64 * 64, 2: 256 * 32 * 32, 3: 256 * 16 * 16,
       4: 256 * 8 * 8, 5: 256 * 4 * 4}
NCH = {1: 16, 2: 32, 3: 64, 4: 128, 5: 1}


def _build(taps=()):
    import contextlib
    import concourse.bass as bass
    import concourse.mybir as mybir
    from concourse.tile import TileContext

    F32 = mybir.dt.float32
    BF16 = mybir.dt.bfloat16
    A = mybir.AluOpType
    AF = mybir.ActivationFunctionType
    AX = mybir.AxisListType

    nc = bass.Bass(num_devices=NCORES)

    p1_d = nc.dram_tensor("p1", [128, 16384], F32, kind="ExternalInput")
    w1rep_d = nc.dram_tensor("w1rep", [128, 16], F32, kind="ExternalInput")
    sw1_d = nc.dram_tensor("sw1", [128, 1], F32, kind="ExternalInput")
    wsc_cols = {2: 128, 3: 192, 4: 1024, 5: 16}
    wsc_d = {l: nc.dram_tensor(f"w{l}sc", [128, wsc_cols[l]], F32, kind="ExternalInput")
             for l in (2, 3, 4, 5)}
    swb_d = {l: nc.dram_tensor(f"sw{l}", [128 if l < 5 else 1, 1], F32, kind="ExternalInput")
             for l in (2, 3, 4, 5)}
    gb_d = {l: nc.dram_tensor(f"gb{l}", [NCH[l], 2], F32, kind="ExternalInput")
            for l in (1, 2, 3, 4, 5)}
    out_d = nc.dram_tensor("out", [1, 512], F32, kind="ExternalOutput")
    tap_d = {}
    for t in taps:
        shp = {"raw1": [128, 16384], "raw2": [128, 8192], "raw3": [128, 4096],
               "raw4": [128, 2048], "raw5": [1, 512]}[t]
        tap_d[t] = nc.dram_tensor("tap_" + t, shp, F32 if t == "raw5" else BF16,
                                  kind="ExternalOutput")

    cc_in = {l: nc.dram_tensor(f"cci{l}", [NCH[l], 2], F32, kind="Internal")
             for l in (1, 2, 3, 4, 5)}
    cc_out = {l: nc.dram_tensor(f"cco{l}", [NCH[l], 2], F32, kind="Internal",
                                addr_space="Shared")
              for l in (1, 2, 3, 4, 5)}

    # inline constants
    ohg1 = np.zeros((128, 256), np.float32)
    bd16 = np.zeros((128, 128), np.float32)
    for k in range(128):
        ohg1[k, 128 + 16 * (k // 16)] = 2.0
        bd16[k, 16 * (k // 16):16 * (k // 16) + 16] = -1.0
    oh2b = np.zeros((128, 256), BF); oh2b[:, 128] = BF(2.0)
    on32 = np.zeros((128, 256), BF); on32[:, 128:160] = BF(-1.0)
    on64 = np.zeros((128, 256), BF); on64[:, 128:192] = BF(-1.0)
    neg1 = np.full((128, 128), -1.0, BF)
    ohl2 = np.zeros((128, 256), BF)
    onl2 = np.zeros((128, 256), BF)
    for k in range(128):
        par = (k % 32) // 16
        ohl2[k, 128 + 2 * par] = BF(2.0)
        for c in range(32):
            onl2[k, 128 + 4 * c + 2 * par] = BF(-1.0)
    sfm, repm = {}, {}
    for l, C in ((1, 16), (2, 32), (3, 64)):
        m = np.zeros((128, C), np.float32)
        r = np.zeros((C, 128), np.float32)
        for k in range(128):
            c = (k // 4) if l == 2 else (k % C)
            m[k, c] = 1.0
            r[c, k] = 1.0
        sfm[l], repm[l] = m, r
    inl = lambda nm, a: nc.inline_tensor(np.ascontiguousarray(a), name=nm)
    ohg1_t, bd16_t = inl("c_ohg1", ohg1), inl("c_bd16", bd16)
    oh2b_t, on32_t = inl("c_oh2b", oh2b), inl("c_on32", on32)
    on64_t, neg1_t = inl("c_on64", on64), inl("c_neg1", neg1)
    ohl2_t, onl2_t = inl("c_ohl2", ohl2), inl("c_onl2", onl2)
    sf_t = {l: inl(f"c_sf{l}", sfm[l]) for l in sfm}
    rep_t = {l: inl(f"c_rep{l}", repm[l]) for l in repm}

    with TileContext(nc) as tc:
        with contextlib.ExitStack() as ctx:
            cp = ctx.enter_context(tc.tile_pool(name="consts", bufs=1))
            hp = ctx.enter_context(tc.tile_pool(name="hpads", bufs=1))
            sp = ctx.enter_context(tc.tile_pool(name="small", bufs=1))
            scratch = ctx.enter_context(tc.tile_pool(name="scratch", bufs=2))
            ps = ctx.enter_context(tc.tile_pool(name="psum", bufs=6, space="PSUM"))
            ps2 = ctx.enter_context(tc.tile_pool(name="psum2", bufs=2, space="PSUM"))

            def load_const(tag, dram, shape, dtype):
                t = cp.tile(shape, dtype, name=tag, tag=tag)
                nc.sync.dma_start(t[:], dram[:])
                return t

            ohg1_s = load_const("ohg1", ohg1_t, [128, 256], F32)
            bd16_s = load_const("bd16", bd16_t, [128, 128], F32)
            oh2b_s = load_const("oh2b", oh2b_t, [128, 256], BF16)
            on32_s = load_const("on32", on32_t, [128, 256], BF16)
            on64_s = load_const("on64", on64_t, [128, 256], BF16)
            neg1_s = load_const("neg1", neg1_t, [128, 128], BF16)
            ohl2_s = load_const("ohl2", ohl2_t, [128, 256], BF16)
            onl2_s = load_const("onl2", onl2_t, [128, 256], BF16)
            sf_s = {l: load_const(f"sf{l}", sf_t[l], [128, NCH[l]], F32) for l in sfm}
            rep_s = {l: load_const(f"rep{l}", rep_t[l], [NCH[l], 128], F32) for l in repm}
            w1rep_s = load_const("w1rep", w1rep_d, [128, 16], F32)
            sw1_s = load_const("sw1", sw1_d, [128, 1], F32)
            wsc_s = {l: load_const(f"wsc{l}", wsc_d[l], [128, wsc_cols[l]], F32)
                     for l in (2, 3, 4, 5)}
            swb_s = {l: load_const(f"swb{l}", swb_d[l], [128 if l < 5 else 1, 1], F32)
                     for l in (2, 3, 4, 5)}
            gb_s = {l: load_const(f"gb{l}", gb_d[l], [NCH[l], 2], F32)
                    for l in (1, 2, 3, 4, 5)}

            h1p = hp.tile([128, 4, 66, 66], BF16, name="h1p", tag="h1p")
            h2p = hp.tile([128, 8, 34, 34], BF16, name="h2p", tag="h2p")
            h3p = hp.tile([128, 16, 18, 18], BF16, name="h3p", tag="h3p")
            h4p = hp.tile([128, 32, 10, 10], BF16, name="h4p", tag="h4p")
            for t in (h1p, h2p, h3p, h4p):
                nc.gpsimd.memset(t[:], 0.0)

            st_s = {l: sp.tile([128 if l < 5 else 1, LCFG[l]["nst"] if l > 1 else 32],
                               F32, name=f"sts{l}", tag=f"sts{l}") for l in (1, 2, 3, 4, 5)}
            st_q = {l: sp.tile([128 if l < 5 else 1, LCFG[l]["nst"] if l > 1 else 32],
                               F32, name=f"stq{l}", tag=f"stq{l}") for l in (1, 2, 3, 4, 5)}

            def bn_coeffs(l):
                C = NCH[l]
                R = 128 if l < 5 else 1
                stf = sp.tile([R, 2], F32, name=f"stf{l}", tag=f"stf{l}")
                nc.vector.tensor_reduce(stf[:, 0:1], st_s[l][:], AX.X, A.add)
                nc.vector.tensor_reduce(stf[:, 1:2], st_q[l][:], AX.X, A.add)
                stc = sp.tile([C, 2], F32, name=f"stc{l}", tag=f"stc{l}")
                if l in sf_s:
                    psf = ps2.tile([C, 2], F32, name="paux", tag="paux")
                    nc.tensor.matmul(psf[:], sf_s[l][:], stf[:], start=True, stop=True)
                    nc.scalar.copy(stc[:], psf[:])
                else:
                    nc.vector.tensor_copy(stc[:], stf[:])
                nc.sync.dma_start(cc_in[l][:], stc[:])
                nc.gpsimd.collective_compute(
                    "AllReduce", A.add, replica_groups=[list(range(NCORES))],
                    ins=[cc_in[l][:]], outs=[cc_out[l][:]])
                nb = sp.tile([R, 4], F32, name=f"nb{l}", tag=f"nb{l}")
                if l in rep_s:
                    rr = sp.tile([C, 4], F32, name=f"rr{l}", tag=f"rr{l}")
                    nc.sync.dma_start(rr[:, 0:2], cc_out[l][:])
                    nc.vector.tensor_copy(rr[:, 2:4], gb_s[l][:])
                    prr = ps2.tile([128, 4], F32, name="paux", tag="paux")
                    nc.tensor.matmul(prr[:], rep_s[l][:], rr[:], start=True, stop=True)
                    nc.scalar.copy(nb[:], prr[:])
                else:
                    nc.sync.dma_start(nb[:, 0:2], cc_out[l][:])
                    nc.vector.tensor_copy(nb[:, 2:4], gb_s[l][:])
                ic = 1.0 / CNT[l]
                mS = sp.tile([R, 1], F32, name=f"mS{l}", tag=f"mS{l}")
                v = sp.tile([R, 1], F32, name=f"v{l}", tag=f"v{l}")
                nc.vector.tensor_scalar_mul(mS[:], nb[:, 0:1], ic)
                nc.vector.tensor_tensor(v[:], mS[:], mS[:], A.mult)
                mQ = sp.tile([R, 1], F32, name=f"mQ{l}", tag=f"mQ{l}")
                nc.vector.tensor_scalar_mul(mQ[:], nb[:, 1:2], ic)
                nc.vector.tensor_tensor(v[:], mQ[:], v[:], A.subtract)
                nc.vector.tensor_scalar_add(v[:], v[:], EPS)
                y0 = sp.tile([R, 1], F32, name=f"y0{l}", tag=f"y0{l}")
                nc.scalar.activation(y0[:], v[:], AF.Sqrt)
                r0 = sp.tile([R, 1], F32, name=f"r0{l}", tag=f"r0{l}")
                nc.vector.reciprocal(r0[:], y0[:])
                t0 = sp.tile([R, 1], F32, name=f"t0{l}", tag=f"t0{l}")
                nc.vector.tensor_tensor(t0[:], v[:], r0[:], A.mult)
                nc.vector.tensor_tensor(t0[:], y0[:], t0[:], A.add)
                nc.vector.tensor_scalar_mul(t0[:], t0[:], 0.5)
                rsq = sp.tile([R, 1], F32, name=f"rsq{l}", tag=f"rsq{l}")
                nc.vector.reciprocal(rsq[:], t0[:])
                a = sp.tile([R, 1], F32, name=f"a{l}", tag=f"a{l}")
                nc.vector.tensor_tensor(a[:], nb[:, 2:3], rsq[:], A.mult)
                c = sp.tile([R, 1], F32, name=f"c{l}", tag=f"c{l}")
                nc.vector.tensor_tensor(c[:], mS[:], a[:], A.mult)
                nc.vector.tensor_tensor(c[:], nb[:, 3:4], c[:], A.subtract)
                return a, c

            # ---------------- Layer 1 ----------------
            with tc.tile_pool(name="l1raw", bufs=1) as rp1, \
                 tc.tile_pool(name="l1p", bufs=2) as pp1, \
                 tc.tile_pool(name="l1d", bufs=3) as dp1:
                raw1 = rp1.tile([128, 16384], BF16, name="raw1", tag="raw1")
                for ch in range(8):
                    p1c = pp1.tile([128, 2048], F32, name="p1c", tag="p1c")
                    nc.sync.dma_start(p1c[:], p1_d[:, ch * 2048:(ch + 1) * 2048])
                    pts = [ps.tile([128, 512], F32, name="pmain", tag="pmain") for _ in range(4)]
                    for tt in range(4):
                        nc.tensor.matmul(pts[tt][:], bd16_s[:],
                                         p1c[:, tt * 512:(tt + 1) * 512],
                                         start=True, stop=False)
                    for co in range(16):
                        d1 = dp1.tile([128, 2048], F32, name="d1", tag="d1")
                        nc.vector.tensor_scalar(d1[:], p1c[:], w1rep_s[:, co:co + 1],
                                                0.0, A.subtract, A.min)
                        for tt in range(4):
                            nc.tensor.matmul(pts[tt][:],
                                             ohg1_s[:, 128 - co:256 - co],
                                             d1[:, tt * 512:(tt + 1) * 512],
                                             start=False, stop=(co == 15))
                    for tt in range(4):
                        t = ch * 4 + tt
                        nc.scalar.activation(raw1[:, t * 512:(t + 1) * 512], pts[tt][:],
                                             AF.Identity, bias=sw1_s[:, 0:1],
                                             accum_out=st_s[1][:, t:t + 1])
                        sq = scratch.tile([128, 512], F32, name="sq1", tag="sq1")
                        nc.scalar.activation(sq[:], raw1[:, t * 512:(t + 1) * 512],
                                             AF.Square, accum_out=st_q[1][:, t:t + 1])

                a1, c1 = bn_coeffs(1)
                nc.scalar.activation(raw1[:], raw1[:], AF.Prelu,
                                     bias=c1[:, 0:1], scale=a1[:, 0:1], alpha=SLOPE)
                if "raw1" in tap_d:
                    nc.sync.dma_start(tap_d["raw1"][:], raw1[:])
                for t in range(32):
                    base = 32 * (t // 8) + 16 * (t % 2)
                    for g in range(8):
                        nc.sync.dma_start(
                            h1p[base:base + 16, (t % 8) // 2,
                                1 + 8 * g:9 + 8 * g, 1:65],
                            raw1[16 * g:16 * g + 16, t * 512:(t + 1) * 512])

            # ---------------- Layers 2-5 ----------------
            def run_layer(l, src_pad, raw_t, pool_p, pool_d):
                cfg = LCFG[l]
                Ci, Co, K, Ho = cfg["Ci"], cfg["Co"], cfg["K"], cfg["Ho"]
                Wo, rowmod, dparts, npg = Ho, cfg["rowmod"], cfg["dparts"], cfg["npg"]
                ck_n = cfg["ck_n"]
                ck_pos = ck_n * Ho * Wo
                nsub = ck_pos // 512
                nblk = 128 // rowmod
                n_pt = max(1, nsub // nblk)
                onesb = {32: on32_s, 64: on64_s, 128: neg1_s}[rowmod]
                for ch in range(cfg["n_chunks"]):
                    ptiles = []
                    for dt, dk in enumerate(dparts):
                        nk = dk // Ci
                        pt_ = pool_p.tile([dk, ck_n, Ho, Wo], BF16, name=f"p{l}_{dt}", tag=f"p{l}_{dt}")
                        for kk in range(nk):
                            khkw = sum(d // Ci for d in dparts[:dt]) + kk
                            kh, kw = khkw // K, khkw % K
                            n0 = ch * ck_n
                            for gg in range(n0 // npg, (n0 + ck_n - 1) // npg + 1):
                                na = max(n0, gg * npg)
                                nb_ = min(n0 + ck_n, (gg + 1) * npg)
                                if na >= nb_:
                                    continue
                                nc.scalar.copy(
                                    pt_[kk * Ci:(kk + 1) * Ci, na - n0:nb_ - n0, :, :],
                                    src_pad[Ci * gg:Ci * gg + Ci,
                                            na - gg * npg:nb_ - gg * npg,
                                            kh:kh + 2 * Ho - 1:2,
                                            kw:kw + 2 * Wo - 1:2])
                        ptiles.append(pt_)
                    pts = [ps.tile([128, 512], F32, name="pmain", tag="pmain") for _ in range(n_pt)]
                    first = [True] * n_pt
                    for dt, dk in enumerate(dparts):
                        pvf = ptiles[dt][:].rearrange("p a b c -> p (a b c)")
                        for s in range(nsub):
                            tt, j = s // nblk, s % nblk
                            lhs = (onesb[0:dk, :] if rowmod == 128 else
                                   onesb[0:dk, 128 - rowmod * j:256 - rowmod * j])
                            nc.tensor.matmul(pts[tt][:], lhs,
                                             pvf[:, s * 512:(s + 1) * 512],
                                             start=first[tt], stop=False)
                            first[tt] = False
                    if Co == 128:
                        # col-tiled: 4 concurrent M=32 matmuls in distinct
                        # 32-col strips, one per co quarter
                        for rr in range(32):
                            for dt, dk in enumerate(dparts):
                                dls = []
                                for q in range(4):
                                    co = rr + 32 * q
                                    dl = pool_d.tile([dk, ck_n, Ho, Wo], BF16,
                                                     name=f"dl{l}", tag=f"dl{l}")
                                    nc.vector.tensor_scalar(
                                        dl[:], ptiles[dt][:],
                                        wsc_s[l][0:dk, dt * Co + co:dt * Co + co + 1],
                                        0.0, A.subtract, A.min)
                                    dls.append(dl[:].rearrange("p a b c -> p (a b c)"))
                                last = (rr == 31) and (dt == len(dparts) - 1)
                                for s in range(nsub):
                                    for q in range(4):
                                        co = rr + 32 * q
                                        nc.tensor.matmul(
                                            pts[s][:],
                                            oh2b_s[0:dk, 128 - co:256 - co],
                                            dls[q][:, s * 512:(s + 1) * 512],
                                            start=False, stop=(last and q == 3))
                    elif l == 3:
                        for rr in range(32):
                            for dt, dk in enumerate(dparts):
                                dls = []
                                for h in (0, 1):
                                    co = rr + 32 * h
                                    dl = pool_d.tile([dk, ck_n, Ho, Wo], BF16,
                                                     name=f"dl{l}", tag=f"dl{l}")
                                    nc.vector.tensor_scalar(
                                        dl[:], ptiles[dt][:],
                                        wsc_s[l][0:dk, dt * Co + co:dt * Co + co + 1],
                                        0.0, A.subtract, A.min)
                                    dls.append(dl[:].rearrange("p a b c -> p (a b c)"))
                                last = (rr == 31) and (dt == len(dparts) - 1)
                                for s in range(nsub):
                                    tt, j = s // 2, s % 2
                                    for h in (0, 1):
                                        r = 64 * j + 32 * h + rr
                                        nc.tensor.matmul(
                                            pts[tt][:],
                                            oh2b_s[0:dk, 128 - r:256 - r],
                                            dls[h][:, s * 512:(s + 1) * 512],
                                            start=False, stop=(last and j == 1 and h == 1))
                    else:
                        for co in range(Co):
                            for dt, dk in enumerate(dparts):
                                dl = pool_d.tile([dk, ck_n, Ho, Wo], BF16, name=f"dl{l}", tag=f"dl{l}")
                                nc.vector.tensor_scalar(
                                    dl[:], ptiles[dt][:],
                                    wsc_s[l][0:dk, dt * Co + co:dt * Co + co + 1],
                                    0.0, A.subtract, A.min)
                                dlf = dl[:].rearrange("p a b c -> p (a b c)")
                                last = (co == Co - 1) and (dt == len(dparts) - 1)
                                for s in range(nsub):
                                    tt, j = s // nblk, s % nblk
                                    r = rowmod * j + co if rowmod < 128 else co
                                    nc.tensor.matmul(
                                        pts[tt][:], oh2b_s[0:dk, 128 - r:256 - r],
                                        dlf[:, s * 512:(s + 1) * 512],
                                        start=False, stop=(last and j == nblk - 1))
                    R = 128 if l < 5 else 1
                    for tt in range(n_pt):
                        t = ch * n_pt + tt
                        nc.scalar.activation(
                            raw_t[0:R, t * 512:(t + 1) * 512], pts[tt][0:R, :],
                            AF.Identity, bias=swb_s[l][:, 0:1],
                            accum_out=st_s[l][:, t:t + 1])
                        sq = scratch.tile([R, 512], F32, name=f"sq{l}", tag=f"sq{l}")
                        nc.scalar.activation(sq[:], raw_t[0:R, t * 512:(t + 1) * 512],
                                             AF.Square, accum_out=st_q[l][:, t:t + 1])

            # L2
            with tc.tile_pool(name="l2raw", bufs=1) as rp2, \
                 tc.tile_pool(name="l2p", bufs=2) as pp2, \
                 tc.tile_pool(name="l2d", bufs=4) as dp2:
                raw2 = rp2.tile([128, 8192], BF16, name="raw2", tag="raw2")
                for ch in range(16):
                    ptiles = []
                    for dt in range(4):
                        pt_ = pp2.tile([128, 32, 32], BF16, name=f"p2_{dt}",
                                       tag=f"p2_{dt}")
                        for k4 in range(4):
                            khkw = dt * 4 + k4
                            kh, kw = khkw // 4, khkw % 4
                            nc.scalar.copy(
                                pt_[32 * k4:32 * k4 + 32, :, :],
                                h1p[32 * (ch // 4):32 * (ch // 4) + 32, ch % 4,
                                    kh:kh + 63:2, kw:kw + 63:2])
                        ptiles.append(pt_)
                    pt = ps.tile([128, 512], F32, name="pmain", tag="pmain")
                    first = True
                    for dt in range(4):
                        pvf = ptiles[dt][:].rearrange("p a b -> p (a b)")
                        for hh in (0, 1):
                            nc.tensor.matmul(
                                pt[:], onl2_s[:, 128 - hh:256 - hh],
                                pvf[:, 512 * hh:512 * hh + 512],
                                start=first, stop=False)
                            first = False
                    for r8 in range(8):
                        for dt in range(4):
                            dls = []
                            for q in range(4):
                                co = r8 + 8 * q
                                dl = dp2.tile([128, 32, 32], BF16, name="dl2", tag="dl2")
                                nc.vector.tensor_scalar(
                                    dl[:], ptiles[dt][:],
                                    wsc_s[2][:, dt * 32 + co:dt * 32 + co + 1],
                                    0.0, A.subtract, A.min)
                                dls.append(dl[:].rearrange("p a b -> p (a b)"))
                            last = (r8 == 7) and (dt == 3)
                            for hh in (0, 1):
                                for q in range(4):
                                    co = r8 + 8 * q
                                    off = 4 * co + hh
                                    nc.tensor.matmul(
                                        pt[:],
                                        ohl2_s[:, 128 - off:256 - off],
                                        dls[q][:, 512 * hh:512 * hh + 512],
                                        start=False, stop=(last and hh == 1 and q == 3))
                    nc.scalar.activation(raw2[:, ch * 512:(ch + 1) * 512], pt[:],
                                         AF.Identity, bias=swb_s[2][:, 0:1],
                                         accum_out=st_s[2][:, ch:ch + 1])
                    sq = scratch.tile([128, 512], F32, name="sq2", tag="sq2")
                    nc.scalar.activation(sq[:], raw2[:, ch * 512:(ch + 1) * 512],
                                         AF.Square, accum_out=st_q[2][:, ch:ch + 1])
                a2, c2 = bn_coeffs(2)
                nc.scalar.activation(raw2[:], raw2[:], AF.Prelu,
                                     bias=c2[:, 0:1], scale=a2[:, 0:1], alpha=SLOPE)
                if "raw2" in tap_d:
                    nc.sync.dma_start(tap_d["raw2"][:], raw2[:])
                for t in range(16):
                    for j in range(4):
                        pos0 = t * 2048 + j * 512
                        n, hh = pos0 // 1024, (pos0 % 1024) // 512
                        nc.sync.dma_start(
                            h2p[32 * (n // 8):32 * (n // 8) + 32, n % 8,
                                1 + 16 * hh:17 + 16 * hh, 1:33],
                            raw2[j:128:4, t * 512:(t + 1) * 512])

            # L3
            with tc.tile_pool(name="l3raw", bufs=1) as rp3, \
                 tc.tile_pool(name="l3p", bufs=2) as pp3, \
                 tc.tile_pool(name="l3d", bufs=4) as dp3:
                raw3 = rp3.tile([128, 4096], BF16, name="raw3", tag="raw3")
                run_layer(3, h2p, raw3, pp3, dp3)
                a3, c3 = bn_coeffs(3)
                nc.scalar.activation(raw3[:], raw3[:], AF.Prelu,
                                     bias=c3[:, 0:1], scale=a3[:, 0:1], alpha=SLOPE)
                if "raw3" in tap_d:
                    nc.sync.dma_start(tap_d["raw3"][:], raw3[:])
                for t in range(8):
                    for j in range(2):
                        n = (t * 1024 + j * 512) // 256
                        for i in range(2):
                            nc.sync.dma_start(
                                h3p[64 * (n // 16):64 * (n // 16) + 64,
                                    n % 16 + i, 1:17, 1:17],
                                raw3[64 * j:64 * j + 64,
                                     t * 512 + i * 256:t * 512 + (i + 1) * 256])

            # L4
            with tc.tile_pool(name="l4raw", bufs=1) as rp4, \
                 tc.tile_pool(name="l4p", bufs=1) as pp4, \
                 tc.tile_pool(name="l4d", bufs=6) as dp4:
                raw4 = rp4.tile([128, 2048], BF16, name="raw4", tag="raw4")
                run_layer(4, h3p, raw4, pp4, dp4)
                a4, c4 = bn_coeffs(4)
                nc.scalar.activation(raw4[:], raw4[:], AF.Prelu,
                                     bias=c4[:, 0:1], scale=a4[:, 0:1], alpha=SLOPE)
                if "raw4" in tap_d:
                    nc.sync.dma_start(tap_d["raw4"][:], raw4[:])
                for t in range(4):
                    for i in range(8):
                        nc.sync.dma_start(
                            h4p[:, 8 * t + i, 1:9, 1:9],
                            raw4[:, t * 512 + i * 64:t * 512 + (i + 1) * 64])

            # L5
            with tc.tile_pool(name="l5raw", bufs=1) as rp5, \
                 tc.tile_pool(name="l5p", bufs=1) as pp5, \
                 tc.tile_pool(name="l5d", bufs=4) as dp5:
                raw5 = rp5.tile([1, 512], F32, name="raw5", tag="raw5")
                run_layer(5, h4p, raw5, pp5, dp5)
                a5, c5 = bn_coeffs(5)
                out5 = sp.tile([1, 512], F32, name="out5", tag="out5")
                nc.scalar.activation(out5[:], raw5[:], AF.Sigmoid,
                                     bias=c5[:, 0:1], scale=a5[:, 0:1])
                if "raw5" in tap_d:
                    nc.sync.dma_start(tap_d["raw5"][:], raw5[:])
                nc.sync.dma_start(out_d[:], out5[:])

    return nc


def _host_prep(inputs):
    x = np.asarray(inputs["x"], np.float32)
    W = {l: np.asarray(inputs[f"W{l}"], np.float32) for l in (1, 2, 3, 4, 5)}
    g = {l: np.asarray(inputs[f"g{l}"], np.float32) for l in (1, 2, 3, 4, 5)}
    b = {l: np.asarray(inputs[f"b{l}"], np.float32) for l in (1, 2, 3, 4, 5)}

    W1f = W[1].reshape(16, 16)
    shared = {
        "w1rep": np.ascontiguousarray(np.tile(W1f.T, (8, 1)), np.float32),
        "sw1": np.ascontiguousarray(np.tile(W1f.sum(1), 8)[:, None], np.float32),
    }
    for l in (2, 3, 4, 5):
        Wd = W[l].transpose(2, 3, 1, 0).reshape(-1, W[l].shape[0])  # [D, Co]
        D, Co = Wd.shape
        if l == 2:
            wsc = np.zeros((128, 4 * 32), np.float32)
            for k in range(128):
                for dt in range(4):
                    khkw = dt * 4 + k // 32
                    wsc[k, dt * 32:(dt + 1) * 32] = Wd[khkw * 16 + (k % 16), :]
            shared["w2sc"] = wsc
        else:
            ndt = len(LCFG[l]["dparts"])
            Wp = np.zeros((ndt * 128, Co), np.float32)
            Wp[:D] = Wd
            shared[f"w{l}sc"] = np.ascontiguousarray(
                Wp.reshape(ndt, 128, Co).transpose(1, 0, 2).reshape(128, ndt * Co))
        swl = Wd.sum(0)
        if l < 5:
            idx = (lambda k: k // 4) if l == 2 else (lambda k: k % Co)
            shared[f"sw{l}"] = np.asarray(
                [swl[idx(k)] for k in range(128)], np.float32)[:, None]
        else:
            shared[f"sw{l}"] = np.ascontiguousarray(swl[:, None], np.float32)
    for l in (1, 2, 3, 4, 5):
        shared[f"gb{l}"] = np.ascontiguousarray(
            np.stack([g[l].ravel(), b[l].ravel()], 1), np.float32)

    in_maps = []
    for c in range(NCORES):
        xs = x[c * NPC:(c + 1) * NPC, 0]
        xp = np.pad(xs, ((0, 0), (1, 1), (1, 1)))
        s = xp.strides
        win = np.lib.stride_tricks.as_strided(
            xp, (NPC, 64, 64, 4, 4), (s[0], 2 * s[1], 2 * s[2], s[1], s[2]))
        P1 = win.transpose(3, 4, 0, 1, 2).reshape(16, NPC * 4096)
        p1 = np.ascontiguousarray(
            P1.reshape(16, 32, 8, 512).transpose(2, 0, 1, 3).reshape(128, 16384),
            dtype=np.float32)
        m = dict(shared)
        m["p1"] = p1
        in_maps.append(m)
    return in_maps


def _run(inputs, taps=(), **kw):
    _install_bir_fix()
    from concourse.bass_utils import run_bass_kernel_spmd
    key = tuple(sorted(taps))
    if key not in _cache:
        _cache[key] = _build(taps)
    in_maps = _host_prep(inputs)
    return run_bass_kernel_spmd(_cache[key], in_maps, list(range(NCORES)), **kw)


def kernel(**inputs):
    res = _run(inputs)
    out = np.zeros((256, 1, 4, 4), np.float32)
    for c in range(NCORES):
        o = np.asarray(res.results[c]["out"], np.float32).reshape(NPC, 4, 4)
        out[c * NPC:(c + 1) * NPC, 0] = o
    return out

